# Initial kernel scaffold
#
"""Trainium2 Bass kernel for nn_ConditioningEncoder.

Pipeline per position: f0/dur scalar MLPs + phone/midi embedding lookups
-> concat -> Linear(320,256) -> LayerNorm -> ReLU -> Linear(256,256).

Strategy (data parallel over 8 cores, 8192 positions each):
- Host folds the small linears: the f0/dur second-layer weights and the
  embedding tables are pre-multiplied by the corresponding row-blocks of
  proj_w1, so the device only does:
    h = relu(f0*w1+b1 | dur*...) @ fdW  +  onehot(phone) @ phW  +  onehot(midi) @ miW
  with all biases folded into the phone table rows.
- Embedding gathers are one-hot matmuls on the PE (tables are tiny).
- LayerNorm stats via bn_stats/bn_aggr; normalize+ReLU fused into one
  scalar-engine activation (per-partition scale=rstd, bias=-mu*rstd).
- y is transposed for the second matmul via DMA xbar block transposes.
- Output staged in SBUF and written back in 1MB DMAs.
- All constants ride in one bf16 + one f32 tensor (2 DMAs) to keep the
  per-instruction semaphore-wait fan-in low.
"""

import numpy as np
import ml_dtypes
from contextlib import ExitStack

import concourse.bass as bass
import concourse.mybir as mybir
import concourse.tile as tile
from concourse import bacc
from concourse.bass_utils import run_bass_kernel_spmd

BF16 = mybir.dt.bfloat16
F32 = mybir.dt.float32
NCORES = 8
B, T, COND = 16, 4096, 256
NPOS = B * T                     # 65536
PER_CORE = NPOS // NCORES        # 8192
NTILES = PER_CORE // 128         # 64 tiles of 128 positions
SUPER = 4                        # tiles per super-tile (512 positions)
OUT_GROUP = 8                    # tiles per output DMA (1MB)
EPS = 1e-5
BFC_COLS = 1728

_cache = {}


def _build_program(apply_gb: bool):
    per_core = PER_CORE
    ntiles = per_core // 128
    nsuper = ntiles // SUPER

    nc = bacc.Bacc("TRN2", target_bir_lowering=False, debug=False)

    # ---- DRAM I/O ----
    d_fd = nc.dram_tensor("fd", [2, per_core], BF16, kind="ExternalInput")
    d_ph = nc.dram_tensor("ph", [1, per_core], BF16, kind="ExternalInput")
    d_mi = nc.dram_tensor("mi", [1, per_core], BF16, kind="ExternalInput")
    d_bfc = nc.dram_tensor("bfc", [128, BFC_COLS], BF16, kind="ExternalInput")
    d_f32c = nc.dram_tensor("f32c", [128, 2], F32, kind="ExternalInput")
    if apply_gb:
        d_gbc = nc.dram_tensor("g_bc", [128, 256], F32, kind="ExternalInput")
        d_bbc = nc.dram_tensor("b_bc", [128, 256], F32, kind="ExternalInput")
    d_out = nc.dram_tensor("out", [per_core, 256], F32, kind="ExternalOutput")

    with tile.TileContext(nc) as tc, ExitStack() as ctx:
        singles = ctx.enter_context(tc.tile_pool(name="singles", bufs=1))
        sb_oh = ctx.enter_context(tc.tile_pool(name="oh", bufs=2))
        sb_fdh = ctx.enter_context(tc.tile_pool(name="fdh", bufs=2))
        sb_small = ctx.enter_context(tc.tile_pool(name="small", bufs=3))
        sb_y = ctx.enter_context(tc.tile_pool(name="y", bufs=3))
        sb_yt = ctx.enter_context(tc.tile_pool(name="yt", bufs=3))
        sb_out = ctx.enter_context(tc.tile_pool(name="ostage", bufs=2))
        pp_bc = ctx.enter_context(tc.tile_pool(name="pbc", bufs=2, space="PSUM"))
        pp_fd = ctx.enter_context(tc.tile_pool(name="pfd", bufs=1, space="PSUM"))
        pp_h = ctx.enter_context(tc.tile_pool(name="ph_", bufs=2, space="PSUM"))
        pp_o = ctx.enter_context(tc.tile_pool(name="po", bufs=2, space="PSUM"))

        # ---- load inputs/constants into SBUF (few DMAs; low sem fan-in) ----
        s_fd = singles.tile([2, per_core], BF16, tag="c_fd")
        nc.gpsimd.dma_start(out=s_fd[:], in_=d_fd[:])
        s_ph = singles.tile([1, per_core], BF16, tag="c_ph")
        nc.gpsimd.dma_start(out=s_ph[:], in_=d_ph[:])
        s_mi = singles.tile([1, per_core], BF16, tag="c_mi")
        nc.gpsimd.dma_start(out=s_mi[:], in_=d_mi[:])
        s_bfc = singles.tile([128, BFC_COLS], BF16, tag="c_bfc")
        nc.gpsimd.dma_start(out=s_bfc[:], in_=d_bfc[:])
        s_f32c = singles.tile([128, 2], F32, tag="c_f32c")
        nc.gpsimd.dma_start(out=s_f32c[:], in_=d_f32c[:])
        if apply_gb:
            s_gbc = singles.tile([128, 256], F32, tag="c_gbc")
            nc.gpsimd.dma_start(out=s_gbc[:], in_=d_gbc[:])
            s_bbc = singles.tile([128, 256], F32, tag="c_bbc")
            nc.gpsimd.dma_start(out=s_bbc[:], in_=d_bbc[:])
        s_eps = singles.tile([128, 1], F32, tag="eps")
        nc.vector.memset(s_eps, EPS)

        # views into the packed constant tile
        s_fdw = s_bfc[0:64, 0:256]
        s_phw = s_bfc[:, 256:512]
        s_miw = s_bfc[:, 512:768]
        s_w2a = s_bfc[:, 768:1024]
        s_w2b = s_bfc[:, 1024:1280]
        s_b2 = s_bfc[0:1, 1280:1536]
        s_ones = s_bfc[0:1, 1536:1664]
        s_w1 = s_bfc[0:2, 1664:1728]
        s_iota = s_f32c[:, 0:1]
        s_b1 = s_f32c[0:64, 1:2]

        out_r = d_out.ap().rearrange("(gi t p) c -> gi t p c", t=OUT_GROUP, p=128)

        ostage = None
        for st in range(nsuper):
            W = SUPER * 128  # 512
            sl = slice(st * W, (st + 1) * W)

            # broadcast phone/midi values across partitions (outer product w/ ones)
            bc_ph = pp_bc.tile([128, W], F32, tag="bc")
            nc.tensor.matmul(bc_ph[:], s_ones, s_ph[:, sl], start=True, stop=True)
            oh_ph = sb_oh.tile([128, W], BF16, tag="ohp")
            nc.vector.tensor_scalar(
                out=oh_ph[:], in0=bc_ph[:], scalar1=s_iota, scalar2=None,
                op0=mybir.AluOpType.is_equal)
            bc_mi = pp_bc.tile([128, W], F32, tag="bc")
            nc.tensor.matmul(bc_mi[:], s_ones, s_mi[:, sl], start=True, stop=True)
            oh_mi = sb_oh.tile([128, W], BF16, tag="ohm")
            nc.vector.tensor_scalar(
                out=oh_mi[:], in0=bc_mi[:], scalar1=s_iota, scalar2=None,
                op0=mybir.AluOpType.is_equal)

            # f0/dur hidden: outer product then relu+bias
            fdp = pp_fd.tile([64, W], F32, tag="fdp")
            nc.tensor.matmul(fdp[:], s_w1, s_fd[:, sl], start=True, stop=True)
            fdh = sb_fdh.tile([64, W], BF16, tag="fdh")
            nc.scalar.activation(
                out=fdh[:], in_=fdp[:], func=mybir.ActivationFunctionType.Relu,
                bias=s_b1, scale=1.0)

            for pair in range(SUPER // 2):
                hps = pp_h.tile([128, 2, 256], F32, tag="hps")
                for tt in range(2):
                    lo = (pair * 2 + tt) * 128
                    nc.tensor.matmul(hps[:, tt, :], fdh[:, lo:lo + 128], s_fdw,
                                     start=True, stop=False)
                    nc.tensor.matmul(hps[:, tt, :], oh_ph[:, lo:lo + 128], s_phw,
                                     start=False, stop=False)
                    nc.tensor.matmul(hps[:, tt, :], oh_mi[:, lo:lo + 128], s_miw,
                                     start=False, stop=True)
                stats = sb_small.tile([128, 2, 6], F32, tag="stats")
                mv = sb_small.tile([128, 2, 2], F32, tag="mv")
                for tt in range(2):
                    nc.vector.bn_stats(out=stats[:, tt, :], in_=hps[:, tt, :])
                    nc.vector.bn_aggr(out=mv[:, tt, :], in_=stats[:, tt, :])
                # rstd = 1/sqrt(var+eps); nmr = -mu*rstd   (both tiles at once)
                rstd = sb_small.tile([128, 2], F32, tag="rstd")
                nc.scalar.activation(
                    out=rstd[:], in_=mv[:, :, 1], func=mybir.ActivationFunctionType.Sqrt,
                    bias=s_eps[:], scale=1.0)
                nc.vector.reciprocal(out=rstd[:], in_=rstd[:])
                negmu = sb_small.tile([128, 2], F32, tag="negmu")
                nc.vector.tensor_scalar(
                    out=negmu[:], in0=mv[:, :, 0], scalar1=-1.0, scalar2=None,
                    op0=mybir.AluOpType.mult)
                nmr = sb_small.tile([128, 2], F32, tag="nmr")
                nc.vector.tensor_mul(out=nmr[:], in0=negmu[:], in1=rstd[:])

                for tt in range(2):
                    gt = st * SUPER + pair * 2 + tt     # global tile idx
                    og = gt % OUT_GROUP
                    if og == 0:
                        ostage = sb_out.tile([128, OUT_GROUP, 256], F32, tag="ost")

                    y = sb_y.tile([128, 256], BF16, tag="y")
                    if not apply_gb:
                        nc.scalar.activation(
                            out=y[:], in_=hps[:, tt, :],
                            func=mybir.ActivationFunctionType.Relu,
                            bias=nmr[:, tt:tt + 1], scale=rstd[:, tt:tt + 1])
                    else:
                        yn = sb_y.tile([128, 256], F32, tag="yn")
                        nc.scalar.activation(
                            out=yn[:], in_=hps[:, tt, :],
                            func=mybir.ActivationFunctionType.Identity,
                            bias=nmr[:, tt:tt + 1], scale=rstd[:, tt:tt + 1])
                        nc.vector.tensor_mul(out=yn[:], in0=yn[:], in1=s_gbc[:])
                        nc.vector.tensor_add(out=yn[:], in0=yn[:], in1=s_bbc[:])
                        nc.vector.tensor_scalar(
                            out=y[:], in0=yn[:], scalar1=0.0, scalar2=None,
                            op0=mybir.AluOpType.max)

                    yta = sb_yt.tile([128, 128], BF16, tag="yta")
                    ytb = sb_yt.tile([128, 128], BF16, tag="ytb")
                    nc.sync.dma_start_transpose(yta[:], y[:, 0:128])
                    nc.sync.dma_start_transpose(ytb[:], y[:, 128:256])

                    # bias matmul first: its inputs are constants, so the
                    # accumulation group's PSUM waits land on a wait-free op
                    ops = pp_o.tile([128, 256], F32, tag="ops")
                    nc.tensor.matmul(ops[:], s_ones, s_b2, start=True, stop=False)
                    nc.tensor.matmul(ops[:], yta[:], s_w2a, start=False, stop=False)
                    nc.tensor.matmul(ops[:], ytb[:], s_w2b, start=False, stop=True)

                    if tt == 0:
                        nc.vector.tensor_copy(out=ostage[:, og, :], in_=ops[:])
                    else:
                        nc.scalar.copy(out=ostage[:, og, :], in_=ops[:])

                    if og == OUT_GROUP - 1:
                        gi = gt // OUT_GROUP
                        nc.gpsimd.dma_start(
                            out=out_r[gi].rearrange("t p c -> p t c"),
                            in_=ostage[:])
    nc.compile()
    return nc


def _prep(inputs):
    """Host-side constant folding. Returns (apply_gb, per-core input maps)."""
    f0 = np.ascontiguousarray(inputs["f0"], dtype=np.float32)
    dur = np.ascontiguousarray(inputs["duration"], dtype=np.float32)
    phone = np.asarray(inputs["phone"])
    midi = np.asarray(inputs["midi"])

    w1f, b1f = np.asarray(inputs["f0_w1"], np.float32), np.asarray(inputs["f0_b1"], np.float32)
    w2f, b2f = np.asarray(inputs["f0_w2"], np.float32), np.asarray(inputs["f0_b2"], np.float32)
    w1d, b1d = np.asarray(inputs["dur_w1"], np.float32), np.asarray(inputs["dur_b1"], np.float32)
    w2d, b2d = np.asarray(inputs["dur_w2"], np.float32), np.asarray(inputs["dur_b2"], np.float32)
    pht = np.asarray(inputs["phone_table"], np.float32)
    mit = np.asarray(inputs["midi_table"], np.float32)
    W1, pb1 = np.asarray(inputs["proj_w1"], np.float32), np.asarray(inputs["proj_b1"], np.float32)
    ln_g, ln_b = np.asarray(inputs["ln_g"], np.float32), np.asarray(inputs["ln_b"], np.float32)
    W2, pb2 = np.asarray(inputs["proj_w2"], np.float32), np.asarray(inputs["proj_b2"], np.float32)

    W1_f0, W1_ph = W1[0:64], W1[64:192]
    W1_mi, W1_du = W1[192:256], W1[256:320]

    fdw = np.vstack([w2f @ W1_f0, w2d @ W1_du])                      # [64,256]
    bias_total = pb1 + b2f @ W1_f0 + b2d @ W1_du                     # [256]
    phw = np.zeros((128, 256), np.float32)
    phw[: pht.shape[0]] = pht @ W1_ph + bias_total
    miw = mit @ W1_mi                                                # [128,256]

    fold_g = bool((ln_g > 0).all() and (ln_b == 0).all())
    apply_gb = not fold_g
    W2e = (ln_g[:, None] * W2) if fold_g else W2

    bf = ml_dtypes.bfloat16
    bfc = np.zeros((128, BFC_COLS), np.float32)
    bfc[0:64, 0:256] = fdw
    bfc[:, 256:512] = phw
    bfc[:, 512:768] = miw
    bfc[:, 768:1024] = W2e[0:128]
    bfc[:, 1024:1280] = W2e[128:256]
    bfc[0, 1280:1536] = pb2
    bfc[0, 1536:1664] = 1.0
    bfc[0, 1664:1696] = w1f[0]
    bfc[1, 1696:1728] = w1d[0]
    f32c = np.zeros((128, 2), np.float32)
    f32c[:, 0] = np.arange(128)
    f32c[0:64, 1] = np.concatenate([b1f, b1d])

    per_core = f0.size // NCORES
    consts = {"bfc": bfc.astype(bf), "f32c": f32c}
    if apply_gb:
        consts["g_bc"] = np.broadcast_to(ln_g, (128, 256)).astype(np.float32).copy()
        consts["b_bc"] = np.broadcast_to(ln_b, (128, 256)).astype(np.float32).copy()

    f0v = f0.reshape(NCORES, per_core)
    durv = dur.reshape(NCORES, per_core)
    phv = phone.reshape(NCORES, per_core).astype(bf)
    miv = midi.reshape(NCORES, per_core).astype(bf)

    in_maps = []
    for c in range(NCORES):
        m = dict(consts)
        m["fd"] = np.stack([f0v[c], durv[c]]).astype(bf)
        m["ph"] = phv[c].reshape(1, per_core)
        m["mi"] = miv[c].reshape(1, per_core)
        in_maps.append(m)
    return apply_gb, in_maps


def kernel(**inputs) -> np.ndarray:
    apply_gb, in_maps = _prep(inputs)
    if apply_gb not in _cache:
        _cache[apply_gb] = _build_program(apply_gb)
    nc = _cache[apply_gb]
    res = run_bass_kernel_spmd(nc, in_maps, core_ids=list(range(NCORES)))
    out = np.concatenate([r["out"] for r in res.results], axis=0)
    return out.reshape(B, T, COND).astype(np.float32)



# revision 13
# speedup vs baseline: 1.9406x; 1.9406x over previous
"""Trainium2 Bass kernel for nn_ConditioningEncoder.

Per position: f0/dur scalar MLPs + phone/midi embedding lookups
-> concat -> Linear(320,256) -> LayerNorm -> ReLU -> Linear(256,256).

Strategy (data parallel over 8 cores, 8192 positions each):
- Host folds the small linears INTO a single combined embedding table:
    comb[phone*128 + midi] = phone_table[phone] @ W1_ph
                           + midi_table[midi] @ W1_mi + all_biases
  (12800 x 256 bf16, 512B rows). The device fetches it with ONE
  dma_gather stream (4 chunks of 2048 indices) instead of one-hot
  matmuls -- no broadcasts, no is_equal, no index compute on device.
- The f0/dur contribution is a PE outer product + tiny matmul; the
  gathered rows are accumulated on top IN PSUM via an identity matmul,
  so LayerNorm stats read exact f32 PSUM via bn_stats (one op per
  2 tiles).
- y = relu(h - mu) (rstd deferred); rstd is applied as a per-partition
  scale fused into the PSUM->SBUF copy after the second matmul.
- y is transposed for the second matmul with PE transpose matmuls
  (~65ns) instead of 1.2us DMA xbar transposes.
- Output is written per-tile as bf16 (host casts to f32).
- Software-pipelined: PE stream for super-tile N is
  [fd-outer(N), inject x4(N), fd-mm x4(N), yT x8(N-1), out x8(N-1)]
  so the tensor engine never waits on the LayerNorm chain.
- Element-wise work is spread over DVE / Scalar / GpSimd.
"""

import numpy as np
import ml_dtypes
from contextlib import ExitStack

import concourse.bass as bass
import concourse.mybir as mybir
import concourse.tile as tile
from concourse import bacc, library_config
from concourse.bass_utils import run_bass_kernel_spmd

BF16 = mybir.dt.bfloat16
F32 = mybir.dt.float32
I16 = mybir.dt.int16
AF = mybir.ActivationFunctionType
ALU = mybir.AluOpType

NCORES = 8
B, T, COND = 16, 4096, 256
NPOS = B * T                      # 65536
PER_CORE = NPOS // NCORES         # 8192
NTILES = PER_CORE // 128          # 64
SUPER = 4                         # tiles per super (512 positions)
NSUPER = NTILES // SUPER          # 16
GCHUNK = 1024                     # positions per dma_gather
NGATH = PER_CORE // GCHUNK        # 4
EPS = 1e-5
BFC_COLS = 1216

_cache = {}


def _build_program(apply_gb: bool, apply_pb2: bool):
    nc = bacc.Bacc("TRN2", target_bir_lowering=False, debug=False)

    d_tab = nc.dram_tensor("tab", [12800, 256], BF16, kind="ExternalInput")
    d_idx = nc.dram_tensor("idx", [128, PER_CORE // 16], I16, kind="ExternalInput")
    d_fd = nc.dram_tensor("fd", [2, PER_CORE], BF16, kind="ExternalInput")
    d_bfc = nc.dram_tensor("bfc", [128, BFC_COLS], BF16, kind="ExternalInput")
    d_f32c = nc.dram_tensor("f32c", [128, 2], F32, kind="ExternalInput")
    if apply_gb:
        d_gbc = nc.dram_tensor("g_bc", [128, 256], F32, kind="ExternalInput")
        d_bbc = nc.dram_tensor("b_bc", [128, 256], F32, kind="ExternalInput")
    if apply_pb2:
        d_pb2 = nc.dram_tensor("pb2_bc", [128, 256], F32, kind="ExternalInput")
    d_out = nc.dram_tensor("out", [NTILES, 128, 256], BF16, kind="ExternalOutput")
    out_ap = d_out.ap()

    with tile.TileContext(nc) as tc, ExitStack() as ctx:
        singles = ctx.enter_context(tc.tile_pool(name="singles", bufs=1))
        sb_fdh = ctx.enter_context(tc.tile_pool(name="fdh", bufs=2))
        sb_small = ctx.enter_context(tc.tile_pool(name="small", bufs=4))
        sb_mv = ctx.enter_context(tc.tile_pool(name="mv", bufs=3))
        sb_y = ctx.enter_context(tc.tile_pool(name="y", bufs=8))
        sb_yt = ctx.enter_context(tc.tile_pool(name="yt", bufs=4))
        sb_o = ctx.enter_context(tc.tile_pool(name="o", bufs=6))
        sb_tmp = ctx.enter_context(tc.tile_pool(name="tmp", bufs=2))
        pp_fd = ctx.enter_context(tc.tile_pool(name="pfd", bufs=1, space="PSUM"))
        pp_h = ctx.enter_context(tc.tile_pool(name="ph_", bufs=4, space="PSUM"))
        pp_t = ctx.enter_context(tc.tile_pool(name="pt", bufs=1, space="PSUM"))
        pp_o = ctx.enter_context(tc.tile_pool(name="po", bufs=2, space="PSUM"))

        # ---- idx + gathers first: the gather chain gates everything ----
        nc.gpsimd.load_library(library_config.mlp)
        s_idx = singles.tile([128, PER_CORE // 16], I16, tag="c_idx")
        nc.sync.dma_start(out=s_idx[:], in_=d_idx[:])
        s_gath = singles.tile([128, NTILES, 256], BF16, tag="gath")
        for c in range(NGATH):
            nc.gpsimd.dma_gather(
                s_gath[:, c * (GCHUNK // 128):(c + 1) * (GCHUNK // 128), :],
                d_tab.ap(),
                s_idx[:, c * (GCHUNK // 16):(c + 1) * (GCHUNK // 16)],
                GCHUNK, GCHUNK, 256)

        # ---- remaining constants / inputs ----
        s_fd = singles.tile([2, PER_CORE], BF16, tag="c_fd")
        nc.sync.dma_start(out=s_fd[:], in_=d_fd[:])
        s_bfc = singles.tile([128, BFC_COLS], BF16, tag="c_bfc")
        nc.sync.dma_start(out=s_bfc[:], in_=d_bfc[:])
        s_f32c = singles.tile([128, 2], F32, tag="c_f32c")
        nc.sync.dma_start(out=s_f32c[:], in_=d_f32c[:])
        if apply_gb:
            s_gbc = singles.tile([128, 256], F32, tag="c_gbc")
            nc.sync.dma_start(out=s_gbc[:], in_=d_gbc[:])
            s_bbc = singles.tile([128, 256], F32, tag="c_bbc")
            nc.sync.dma_start(out=s_bbc[:], in_=d_bbc[:])
        if apply_pb2:
            s_pb2 = singles.tile([128, 256], F32, tag="c_pb2")
            nc.sync.dma_start(out=s_pb2[:], in_=d_pb2[:])

        s_fdw = s_bfc[0:64, 0:256]
        s_w2a = s_bfc[:, 256:512]
        s_w2b = s_bfc[:, 512:768]
        s_id = s_bfc[:, 768:896]
        s_zero = s_bfc[:, 896:1152]
        s_w1 = s_bfc[0:2, 1152:1216]
        s_eps = s_f32c[:, 0:1]
        s_b1 = s_f32c[0:64, 1:2]

        prev = None  # state of super st-1 for the B-stage
        for st in range(NSUPER + 1):
            # ---- B-stage for super st-1 first: its deps are ready, so the
            # PE never head-of-line blocks on the gather-gated A-stage ----
            if prev is not None:
                y_tiles, mv_p, rstd_p, nmr_p, pst = prev
                for tt in range(SUPER):
                    gt = pst * SUPER + tt
                    y = y_tiles[tt]
                    if tt % 2 == 0:
                        ytp = pp_t.tile([128, 2, 256], BF16, tag="ytp")
                        opsp = pp_o.tile([128, 2, 256], F32, tag="ops")
                    j = tt % 2
                    nc.tensor.transpose(ytp[:, j, 0:128], y[:, 0:128], s_id)
                    nc.tensor.transpose(ytp[:, j, 128:256], y[:, 128:256], s_id)
                    ytc = sb_yt.tile([128, 256], BF16, tag="ytc")
                    if tt % 2 == 0:
                        nc.vector.tensor_copy(out=ytc[:], in_=ytp[:, j, :])
                    else:
                        nc.scalar.copy(out=ytc[:], in_=ytp[:, j, :])
                    ops = opsp[:, j, :]
                    nc.tensor.matmul(ops, ytc[:, 0:128], s_w2a,
                                     start=True, stop=False)
                    nc.tensor.matmul(ops, ytc[:, 128:256], s_w2b,
                                     start=False, stop=True)
                    ot = sb_o.tile([128, 256], BF16, tag="ot")
                    if apply_gb:
                        # rstd already applied in y-act
                        if apply_pb2:
                            nc.vector.scalar_tensor_tensor(
                                out=ot[:], in0=ops, scalar=1.0, in1=s_pb2[:],
                                op0=ALU.mult, op1=ALU.add)
                        elif tt % 2 == 0:
                            nc.vector.tensor_copy(out=ot[:], in_=ops)
                        else:
                            nc.scalar.copy(out=ot[:], in_=ops)
                    elif apply_pb2:
                        nc.vector.scalar_tensor_tensor(
                            out=ot[:], in0=ops, scalar=rstd_p[:, tt:tt + 1],
                            in1=s_pb2[:], op0=ALU.mult, op1=ALU.add)
                    elif tt % 2 == 0:
                        nc.vector.tensor_scalar(
                            out=ot[:], in0=ops, scalar1=rstd_p[:, tt:tt + 1],
                            scalar2=None, op0=ALU.mult)
                    else:
                        nc.scalar.mul(ot[:], ops, rstd_p[:, tt:tt + 1])
                    nc.sync.dma_start(out=out_ap[gt], in_=ot[:])

            # ---- A-stage for super st ----
            if st < NSUPER:
                sl = slice(st * 512, (st + 1) * 512)
                # fd outer product -> hidden pre-activation
                fdp = pp_fd.tile([64, 512], F32, tag="fdp")
                nc.tensor.matmul(fdp[:], s_w1, s_fd[:, sl], start=True, stop=True)
                fdh = sb_fdh.tile([64, 512], BF16, tag="fdh")
                nc.scalar.activation(out=fdh[:], in_=fdp[:], func=AF.Relu,
                                     bias=s_b1, scale=1.0)
                # h = gathered(comb) + fdh @ fdw, accumulated in PSUM
                hps_pairs = []
                for pair in range(2):
                    hps = pp_h.tile([128, 2, 256], F32, tag="hps")
                    g0 = st * SUPER + pair * 2
                    nc.tensor.matmul(hps[:], s_id, s_gath[:, g0:g0 + 2, :],
                                     start=True, stop=False,
                                     skip_group_check=True)
                    for j in range(2):
                        tt = pair * 2 + j
                        lo = tt * 128
                        nc.tensor.matmul(hps[:, j, :], fdh[:, lo:lo + 128], s_fdw,
                                         start=False, stop=(j == 1),
                                         skip_group_check=True)
                    hps_pairs.append(hps)
                # LN stats
                mv = sb_mv.tile([128, SUPER, 2], F32, tag="mv")
                for pair in range(2):
                    stats = sb_small.tile([128, 2, 6], F32, tag="stats")
                    for j in range(2):
                        nc.vector.bn_stats(out=stats[:, j, :],
                                           in_=hps_pairs[pair][:, j, :])
                        nc.vector.bn_aggr(out=mv[:, pair * 2 + j, :],
                                          in_=stats[:, j, :])
                sd = sb_mv.tile([128, SUPER], F32, tag="sd")
                nc.scalar.activation(out=sd[:], in_=mv[:, :, 1], func=AF.Sqrt,
                                     bias=s_eps, scale=1.0)
                rstd = sb_mv.tile([128, SUPER], F32, tag="rstd")
                nc.vector.reciprocal(out=rstd[:], in_=sd[:])
                negmu = sb_mv.tile([128, SUPER], F32, tag="negmu")
                nc.scalar.mul(negmu[:], mv[:, :, 0], -1.0)
                nmr = None
                if apply_gb:
                    nmr = sb_mv.tile([128, SUPER], F32, tag="nmr")
                    nc.vector.tensor_mul(out=nmr[:], in0=negmu[:], in1=rstd[:])
                # y = relu(h - mu)  (rstd deferred to the output copy)
                y_tiles = []
                for tt in range(SUPER):
                    hsl = hps_pairs[tt // 2][:, tt % 2, :]
                    y = sb_y.tile([128, 256], BF16, tag="y")
                    if apply_gb:
                        yt_ = sb_tmp.tile([128, 256], F32, tag="ytmp")
                        nc.scalar.activation(out=yt_[:], in_=hsl, func=AF.Identity,
                                             bias=nmr[:, tt:tt + 1],
                                             scale=rstd[:, tt:tt + 1])
                        nc.vector.tensor_mul(out=yt_[:], in0=yt_[:], in1=s_gbc[:])
                        nc.vector.tensor_add(out=yt_[:], in0=yt_[:], in1=s_bbc[:])
                        nc.vector.tensor_scalar(out=y[:], in0=yt_[:], scalar1=0.0,
                                                scalar2=None, op0=ALU.max)
                    elif tt % 2 == 0:
                        # (h - mu) straight off mv; no negmu dependency
                        nc.vector.tensor_scalar(
                            out=y[:], in0=hsl, scalar1=mv[:, tt, 0:1],
                            scalar2=0.0, op0=ALU.subtract, op1=ALU.max)
                    else:
                        nc.scalar.activation(out=y[:], in_=hsl, func=AF.Relu,
                                             bias=negmu[:, tt:tt + 1], scale=1.0)
                    y_tiles.append(y)
                prev = (y_tiles, mv, rstd, nmr, st)
            else:
                prev = None
    nc.compile()
    return nc


def _prep(inputs):
    """Host-side folding. Returns (apply_gb, apply_pb2, per-core input maps)."""
    f0 = np.asarray(inputs["f0"], np.float32)
    dur = np.asarray(inputs["duration"], np.float32)
    phone = np.asarray(inputs["phone"])
    midi = np.asarray(inputs["midi"])

    w1f, b1f = np.asarray(inputs["f0_w1"], np.float32), np.asarray(inputs["f0_b1"], np.float32)
    w2f, b2f = np.asarray(inputs["f0_w2"], np.float32), np.asarray(inputs["f0_b2"], np.float32)
    w1d, b1d = np.asarray(inputs["dur_w1"], np.float32), np.asarray(inputs["dur_b1"], np.float32)
    w2d, b2d = np.asarray(inputs["dur_w2"], np.float32), np.asarray(inputs["dur_b2"], np.float32)
    pht = np.asarray(inputs["phone_table"], np.float32)
    mit = np.asarray(inputs["midi_table"], np.float32)
    W1, pb1 = np.asarray(inputs["proj_w1"], np.float32), np.asarray(inputs["proj_b1"], np.float32)
    ln_g, ln_b = np.asarray(inputs["ln_g"], np.float32), np.asarray(inputs["ln_b"], np.float32)
    W2, pb2 = np.asarray(inputs["proj_w2"], np.float32), np.asarray(inputs["proj_b2"], np.float32)

    W1_f0, W1_ph = W1[0:64], W1[64:192]
    W1_mi, W1_du = W1[192:256], W1[256:320]

    fdw = np.vstack([w2f @ W1_f0, w2d @ W1_du])                      # [64,256]
    bias_total = pb1 + b2f @ W1_f0 + b2d @ W1_du                     # [256]
    ph_part = pht @ W1_ph + bias_total                               # [100,256]
    mi_part = mit @ W1_mi                                            # [128,256]
    comb = ph_part[:, None, :] + mi_part[None, :, :]                 # [100,128,256]

    fold_g = bool((ln_g > 0).all() and (ln_b == 0).all())
    apply_gb = not fold_g
    apply_pb2 = bool((pb2 != 0).any())
    W2e = (ln_g[:, None] * W2) if fold_g else W2

    bf = ml_dtypes.bfloat16
    bfc = np.zeros((128, BFC_COLS), np.float32)
    bfc[0:64, 0:256] = fdw
    bfc[:, 256:512] = W2e[0:128]
    bfc[:, 512:768] = W2e[128:256]
    bfc[:, 768:896] = np.eye(128)
    bfc[0, 1152:1184] = w1f[0]
    bfc[1, 1184:1216] = w1d[0]
    f32c = np.zeros((128, 2), np.float32)
    f32c[:, 0] = EPS
    f32c[0:64, 1] = np.concatenate([b1f, b1d])

    consts = {"tab": comb.reshape(12800, 256).astype(bf),
              "bfc": bfc.astype(bf), "f32c": f32c}
    if apply_gb:
        consts["g_bc"] = np.broadcast_to(ln_g, (128, 256)).astype(np.float32).copy()
        consts["b_bc"] = np.broadcast_to(ln_b, (128, 256)).astype(np.float32).copy()
    if apply_pb2:
        consts["pb2_bc"] = np.broadcast_to(pb2, (128, 256)).astype(np.float32).copy()

    idx_full = (phone.astype(np.int32) * 128 + midi.astype(np.int32)).astype(np.int16)
    idx_full = idx_full.reshape(NCORES, PER_CORE)
    f0v = f0.reshape(NCORES, PER_CORE)
    durv = dur.reshape(NCORES, PER_CORE)

    in_maps = []
    for c in range(NCORES):
        m = dict(consts)
        m["fd"] = np.stack([f0v[c], durv[c]]).astype(bf)
        chunks = []
        for g in range(NGATH):
            w = idx_full[c, g * GCHUNK:(g + 1) * GCHUNK].reshape(GCHUNK // 16, 16).T
            chunks.append(np.tile(w, (8, 1)))            # [128, GCHUNK//16]
        m["idx"] = np.concatenate(chunks, axis=1)        # [128, PER_CORE//16]
        in_maps.append(m)
    return apply_gb, apply_pb2, in_maps


def kernel(**inputs) -> np.ndarray:
    apply_gb, apply_pb2, in_maps = _prep(inputs)
    key = (apply_gb, apply_pb2)
    if key not in _cache:
        _cache[key] = _build_program(apply_gb, apply_pb2)
    nc = _cache[key]
    res = run_bass_kernel_spmd(nc, in_maps, core_ids=list(range(NCORES)))
    out = np.concatenate(
        [r["out"].reshape(PER_CORE, COND) for r in res.results], axis=0)
    return out.reshape(B, T, COND).astype(np.float32)
